# revision 27
# baseline (speedup 1.0000x reference)
"""Multi-head attention forward on 8 Trainium2 NeuronCores (Bass/Tile).

Problem: B=4, T=2048, D=512, H=8, HS=64, fp32.
  q/k/v = einsum('btd,hde->bhte', x, W{q,k,v})
  att   = softmax(q k^T / sqrt(HS))
  out   = (att v) concat-heads @ Wo + bo

Sharding (8 cores): core c -> batch b=c//2, heads hb=4*(c%2)..hb+4
(data parallel on B x tensor parallel on H). Each core computes its 4 heads'
attention and a partial output projection against its 256 rows of Wo; the
host sums the two partials per batch and adds the bias.

On-device dataflow per core (all matmuls in float16: 1 cycle/row on PE,
~1e-3 rel err; PSUM accumulation is fp32):
  phase 1: qT/kT per head-pair [128=2*HS, T] and v [T, 4*(HS+1)] (ones column
           appended per head for the softmax denominator) from xT [D, T].
           Wq/Wk are pre-scaled by 1/16 each on the host, so raw scores come
           out as score/256 and exp() becomes exp(32*s).
  phase 2: per head-pair, per 512-wide t-chunk, loop s-tiles of 128:
           ST[s,t] matmuls (K=HS=64, two heads row-packed at partitions 0/64),
           then exp over the [128, 1024] PSUM tile SPLIT ACROSS ENGINES:
           ScalarE does head j=0 (and both heads on non-split iterations);
           the DVE does head j=1 via two custom ops (cubic minimax poly of
           e^t on |t|<=0.21, then x^32 by 5 squarings; ~1e-3 max rel err).
           AV matmuls accumulate [65, 512] per head (row 0 = sum of exp).
           Normalize: DVE fast-reciprocal of the sum row, partition-broadcast
           on GpSimd, multiply on DVE -> outT [4*HS, T].
  phase 3: y[t,:] = outT.T @ Wo_rows via accumulating matmuls, ScalarE
           PSUM->SBUF fp16 copy, DMA out. (Bias is added on the host.)

With exp split ~56/72 between DVE and ScalarE, the PE matmul stream
(~330K cycles: ST 131K + AV 131K + qkv 48K + oproj 16K) is the critical
path; the emission schedule keeps it saturated: minimal prologue, all
projection work placed as PE filler with per-tile deadlines, exp engines
balanced so neither ever blocks an AV matmul.
"""
import os
import sys

sys.path.insert(0, "/opt/trn_rl_repo")

import numpy as np
from contextlib import ExitStack

import concourse.bacc as bacc
import concourse.tile as tile
from concourse import mybir
from concourse.bass_utils import run_bass_kernel_spmd
from concourse.dve_spec import Spec, Src0, C0, C1, C2, One, lower
from concourse.dve_uop import DveOpSpec
import concourse.dve_ops as dve_ops
from concourse.dve_ops import DveOp, OPS, _SUB_OPCODE_FOR_NAME, _CUSTOM_DVE_ROW_BASE

B, T, D, H, HS = 4, 2048, 512, 8, 64
NCORES = 8
P = 128
HPC = 4  # heads per core
F32 = mybir.dt.float32
F16 = mybir.dt.float16
EXP = mybir.ActivationFunctionType.Exp

# cubic minimax of e^t on [-0.21, 0.21] (covers |score/8| <= 6.7)
EXPC = (1.00001443, 0.50158342, 0.16650198)
# iterations (within each 16-iteration block) whose j=1 exp half runs on DVE
DVE_SI = frozenset({2, 4, 6, 8, 10, 12, 14})


def _ref_exp_cubic(in0, in1, s0, s1, imm2):
    t = np.asarray(in0, dtype=np.float32)
    return ((np.float32(imm2) * t + np.float32(s1)) * t + np.float32(s0)) * t + np.float32(1)


def _ref_sq5(in0, in1, s0, s1, imm2):
    p = np.asarray(in0, dtype=np.float32)
    for _ in range(5):
        p = p * p
    return p


def _register_exp_ops():
    """Install the two custom DVE ops (idempotent across re-imports)."""
    if hasattr(dve_ops, "EXP_CUBIC_ANT"):
        return dve_ops.EXP_CUBIC_ANT, dve_ops.EXP_SQ5_ANT

    def make(name, body, reference):
        spec = Spec(body=body, reference=reference)
        shas = {}
        for ver in ("v3", "v4"):
            shas[ver] = DveOpSpec(
                name=name, opcode=31, uops=lower(spec, ver=ver), rd1_en=False
            ).sha(ver)
        return DveOp(name, spec, subdim=False, uops_sha=shas)

    t = Src0
    body1 = ((C2 * t + C1) * t + C0) * t + One
    s = Src0
    for _ in range(5):
        s = s * s
    op1 = make("EXP_CUBIC_ANT", body1, _ref_exp_cubic)
    op2 = make("EXP_SQ5_ANT", s, _ref_sq5)
    for op in (op1, op2):
        OPS.append(op)
        _SUB_OPCODE_FOR_NAME[op.name] = _CUSTOM_DVE_ROW_BASE + len(OPS) - 1
    dve_ops.EXP_CUBIC_ANT = op1
    dve_ops.EXP_SQ5_ANT = op2
    return op1, op2


OP_EXPC, OP_SQ5 = _register_exp_ops()


def to_fp16(x: np.ndarray) -> np.ndarray:
    return np.ascontiguousarray(np.asarray(x, dtype=np.float32).astype(np.float16))


def _emit(tc, xT, wq, wk, wv, wo, y):
    nc = tc.nc
    with ExitStack() as ctx:
        persist = ctx.enter_context(tc.tile_pool(name="persist", bufs=1))

        # ---- persistent SBUF tiles ----
        xt_sb = [persist.tile([P, T], F16, tag=f"xt{i}", name=f"xt{i}") for i in range(4)]
        wq_sb = [persist.tile([P, 2 * P], F16, tag=f"wq{i}", name=f"wq{i}") for i in range(4)]
        wk_sb = [persist.tile([P, 2 * P], F16, tag=f"wk{i}", name=f"wk{i}") for i in range(4)]
        wv_sb = [persist.tile([P, 2 * P], F16, tag=f"wv{i}", name=f"wv{i}") for i in range(4)]
        wo_sb = [persist.tile([P, D], F16, tag=f"wo{i}", name=f"wo{i}") for i in range(2)]
        q2 = [persist.tile([P, T], F16, tag=f"q2{i}", name=f"q2_{i}") for i in range(2)]
        k2 = [persist.tile([P, T], F16, tag=f"k2{i}", name=f"k2_{i}") for i in range(2)]
        v_sb = [persist.tile([P, HPC * 2 * HS], F16, tag=f"v{i}", name=f"v{i}") for i in range(16)]
        out2 = [persist.tile([P, T], F16, tag=f"o2{i}", name=f"o2_{i}") for i in range(2)]
        ones_v16 = persist.tile([P, HPC], F16, tag="ones_v16")
        warm_in = persist.tile([P, 512], F16, tag="warm_in")

        # PE warm-up: ~3.5us of dummy matmuls during the DMA lead-in flips
        # the HAM clock gate to 2.4 GHz before the real matmuls start.
        with tc.tile_pool(name="ps_warm", bufs=1, space="PSUM") as ps_warm:
            nc.vector.memset(warm_in, 0.5)
            wp = ps_warm.tile([P, 512], F32, tag="warm")
            for _ in range(8):
                nc.tensor.matmul(wp, warm_in[:, 0:P], warm_in, start=True, stop=True)

        # Input DMAs on the three DMA-capable queues (sync/scalar/gpsimd).
        # The four chunk-0 xT tiles lead each queue so the first projection
        # group can start ~2.5us after issue; everything else is ordered by
        # its first consumer's deadline. The scalar queue is kept short so
        # the ACT sequencer is free before the first exp.
        c0 = slice(0, 512)

        def dsl(i):
            return slice(i * P, (i + 1) * P)

        nc.sync.dma_start(out=xt_sb[0][:, c0], in_=xT[dsl(0), c0])
        nc.scalar.dma_start(out=xt_sb[2][:, c0], in_=xT[dsl(2), c0])
        nc.gpsimd.dma_start(out=xt_sb[3][:, c0], in_=xT[dsl(3), c0])
        nc.sync.dma_start(out=xt_sb[1][:, c0], in_=xT[dsl(1), c0])
        nc.scalar.dma_start(out=wq_sb[0], in_=wq[dsl(0), :])
        nc.scalar.dma_start(out=wq_sb[1], in_=wq[dsl(1), :])
        nc.sync.dma_start(out=wq_sb[2], in_=wq[dsl(2), :])
        nc.sync.dma_start(out=wq_sb[3], in_=wq[dsl(3), :])
        for i in range(4):
            nc.gpsimd.dma_start(out=wk_sb[i], in_=wk[dsl(i), :])
        nc.sync.dma_start(out=wv_sb[0], in_=wv[dsl(0), :])
        nc.sync.dma_start(out=wv_sb[1], in_=wv[dsl(1), :])
        nc.gpsimd.dma_start(out=wv_sb[2], in_=wv[dsl(2), :])
        nc.gpsimd.dma_start(out=wv_sb[3], in_=wv[dsl(3), :])
        for tch in range(1, 4):
            csl = slice(tch * 512, (tch + 1) * 512)
            for i in range(4):
                nc.sync.dma_start(out=xt_sb[i][:, csl], in_=xT[dsl(i), csl])
        for i in range(2):
            nc.gpsimd.dma_start(out=wo_sb[i], in_=wo[i * P : (i + 1) * P, :])
        nc.vector.memset(ones_v16, 1.0)

        # One shared PSUM layout: st 2x[128,1024] (4 banks) + 4 general
        # [128,512] slots (tag "av": projection groups AND the AV
        # accumulators) = 8 banks.
        with (
            tc.tile_pool(name="ps_st0", bufs=2, space="PSUM") as ps_st0,
            tc.tile_pool(name="ps_st1", bufs=2, space="PSUM") as ps_st1,
            tc.tile_pool(name="ps_av", bufs=4, space="PSUM") as ps_av,
            tc.tile_pool(name="attp", bufs=6) as attp,
            tc.tile_pool(name="attp0", bufs=4) as attp0,
            tc.tile_pool(name="attp1", bufs=4) as attp1,
            tc.tile_pool(name="dvescr", bufs=3) as dvescr,
            tc.tile_pool(name="nrm", bufs=6) as nrm,
            tc.tile_pool(name="yout", bufs=3) as yout,
        ):
            blocks = [(tq, hp) for tq in range(4) for hp in range(2)]
            avs = [None] * len(blocks)
            # Explicit cross-engine deps where Tile's subtile tracking is
            # unreliable (partition-subrange reads of DVE-written tiles):
            # q/k chunk copies -> ST matmuls, norm multiplies -> projection.
            qk_cp = {}
            nrm_mul = {}
            av_last = {}
            v_cp = {}

            def emit_qk_group(kind, pr, tch):
                # one [128,512] chunk of the q or k projection (4 accumulating
                # matmuls over D, then a cast-copy to fp16 SBUF)
                w_sb, dst = (wq_sb, q2) if kind == "q" else (wk_sb, k2)
                psl = slice(pr * P, (pr + 1) * P)
                tsl = slice(tch * 512, (tch + 1) * 512)
                pt = ps_av.tile(
                    [P, 512], F32, tag="av", name=f"qk{kind}_{pr}_{tch}"
                )
                for di in range(4):
                    nc.tensor.matmul(
                        pt,
                        w_sb[di][:, psl],
                        xt_sb[di][:, tsl],
                        start=(di == 0),
                        stop=(di == 3),
                    )
                qk_cp[(kind, pr, tch)] = nc.vector.tensor_copy(dst[pr][:, tsl], pt)

            def emit_v_group(tt):
                ttsl = slice(tt * P, (tt + 1) * P)
                pv = ps_av.tile([P, 2 * P], F32, tag="av", name=f"pv{tt}")
                for di in range(4):
                    nc.tensor.matmul(
                        pv,
                        xt_sb[di][:, ttsl],
                        wv_sb[di],
                        start=(di == 0),
                        stop=(di == 3),
                    )
                # Per head, the 128 lhsT columns are [ones, 63 junk, v x 64]:
                # the denominator accumulates on partition 0 of the AV tile
                # (read directly by the DVE reciprocal -- the custom op
                # mishandles nonzero partition offsets) and the numerators
                # land at partition 64 (quadrant-aligned for the multiply).
                # The junk columns cost nothing: matmul time depends only on
                # the moving free size, and their accumulator rows are never
                # read.
                v3 = v_sb[tt].rearrange("p (h e) -> p h e", h=HPC)
                c1 = nc.vector.tensor_copy(
                    v3[:, :, HS : 2 * HS], pv.rearrange("p (h e) -> p h e", h=HPC)
                )
                c2 = nc.vector.tensor_copy(v3[:, :, 0], ones_v16)
                v_cp[tt] = (c1, c2)

            def emit_proj(tt):
                # output projection t-tile; PSUM->SBUF fp16 copy on ScalarE
                # (bias is added by the host after the partial-sum gather)
                ttsl = slice(tt * P, (tt + 1) * P)
                tq = tt // 4
                yp = ps_av.tile([P, D], F32, tag="av", name=f"yp{tt}")
                for hp in range(2):
                    mm = nc.tensor.matmul(
                        yp, out2[hp][:, ttsl], wo_sb[hp], start=(hp == 0), stop=(hp == 1)
                    )
                    for j in range(2):
                        dep = nrm_mul.get((hp, tq, j))
                        if dep is not None:
                            tile.add_dep_helper(mm.ins, dep.ins, reason="norm ready")
                ys = yout.tile([P, D], F16, tag="y")
                nc.scalar.copy(ys, yp)
                nc.sync.dma_start(out=y[ttsl, :], in_=ys)

            def emit_norm(b, tail=False):
                # divide the pair's unnormalized outputs by their sums of
                # exp. First drain each accumulator PSUM->SBUF with one DVE
                # copy -- this frees the PSUM slot ~0.8us after the last AV
                # matmul instead of after the whole norm chain, so the next
                # block's accumulators aren't WAR-gated on it. Then: DVE
                # fast reciprocal of the sum row, partition-broadcast on
                # GpSimd, multiply on DVE, all reading the SBUF copy.
                tq, hp = blocks[b]
                av = avs[b]
                tsl = slice(tq * 512, (tq + 1) * 512)
                cps = []
                rcs = []
                for j in range(2):
                    fin = av_last[(b, j)]
                    avc = nrm.tile([HS, 512], F32, tag="avc", name=f"avc{b}_{j}", bufs=4)
                    ci = nc.vector.tensor_copy(avc, av[j][HS : 2 * HS, :])
                    tile.add_dep_helper(ci.ins, fin.ins, reason="av accum done")
                    cps.append(avc)
                    rc = nrm.tile([1, 512], F32, tag="rc", name=f"rc{b}_{j}")
                    ri = nc.vector.reciprocal_approx_fast(rc, av[j][0:1, :])
                    tile.add_dep_helper(ri.ins, fin.ins, reason="av accum done")
                    rcs.append(rc)
                for j in range(2):
                    bco = nrm.tile([HS, 512], F32, tag="bco", name=f"bco{b}_{j}")
                    nc.gpsimd.partition_broadcast(bco, rcs[j])
                    # the multiply reads only SBUF now (avc copy + bco), so
                    # it can run on the otherwise-idle GpSimd instead of
                    # loading the DVE at block boundaries
                    mi = nc.gpsimd.tensor_mul(
                        out2[hp][j * HS : (j + 1) * HS, tsl],
                        cps[j],
                        bco,
                    )
                    nrm_mul[(hp, tq, j)] = mi

            # last exp instruction that read each ps_st1 buffer, by parity:
            # its completion gates the buffer's reuse two iterations later.
            st1_reader = {}
            st_parity = [0]

            def emit_st(hp, tq, si):
                # scores for both heads of the pair, row-packed at
                # partitions 0 / 64 (K=64 each). One PSUM tile PER HEAD so
                # each has exactly ONE exp reader (two readers of one PSUM
                # tile fully serialize in Tile's tracking). The st1 buffer's
                # WAR dep is added explicitly to the FIRST matmul of the
                # pair: Tile then elides the auto-WAR wait on the second,
                # which lets it overlap the first in the other array
                # row-group (~free instead of ~320ns).
                tsl = slice(tq * 512, (tq + 1) * 512)
                ssl = slice(si * P, (si + 1) * P)
                st0 = ps_st0.tile([P, 512], F32, tag="st0", name=f"st0_{hp}_{tq}_{si}")
                st1 = ps_st1.tile([P, 512], F32, tag="st1", name=f"st1_{hp}_{tq}_{si}")
                par = st_parity[0]
                st_parity[0] ^= 1
                prev_rd = st1_reader.get(par)
                for j, stj in enumerate((st0, st1)):
                    hsl = slice(j * HS, (j + 1) * HS)
                    mm = nc.tensor.matmul(
                        stj,
                        k2[hp][hsl, ssl],
                        q2[hp][hsl, tsl],
                        start=True,
                        stop=True,
                    )
                    if j == 0 and prev_rd is not None:
                        tile.add_dep_helper(mm.ins, prev_rd.ins, reason="st1 war")
                    for key in (("q", hp, tq), ("k", hp, si // 4)):
                        cp = qk_cp.get(key)
                        if cp is not None:
                            tile.add_dep_helper(mm.ins, cp.ins, reason="qk chunk")
                return (st0, st1, par)

            # Filler schedule: sched[b][si] = list of thunks issued on the PE
            # (or DVE/GpSimd for norms) inside iteration si of block b,
            # between the hoisted ST(si+1) and the AV(si) matmuls. Every item
            # is placed ahead of its consumer's deadline:
            #   k chunk c of the running pair  -> before ST(4c) issues (iter 4c-1)
            #   v tile si                      -> before AV(si) (iter si)
            #   q/k chunk 0 of the next pair   -> before the next block's ST(0)
            #   q chunk tq                     -> before block (tq,hp) starts
            #   norm of block b-1              -> iter 0 (frees its PSUM pair)
            #   proj tiles of t-chunk tq       -> after both norms of tq
            sched = [dict() for _ in range(len(blocks))]

            def put(b, it, fn, *args):
                sched[b].setdefault(it, []).append((fn, args))

            # block 0: v tiles + remaining k chunks of pair 0, then pair 1
            put(0, 0, emit_v_group, 0)
            put(0, 0, emit_v_group, 1)
            put(0, 0, emit_qk_group, "k", 0, 1)
            put(0, 1, emit_v_group, 2)
            put(0, 1, emit_v_group, 3)
            put(0, 2, emit_v_group, 4)
            put(0, 3, emit_qk_group, "k", 0, 2)
            put(0, 3, emit_v_group, 5)
            put(0, 4, emit_v_group, 6)
            put(0, 4, emit_v_group, 7)
            put(0, 5, emit_v_group, 8)
            put(0, 6, emit_v_group, 9)
            put(0, 7, emit_qk_group, "k", 0, 3)
            put(0, 7, emit_v_group, 10)
            put(0, 8, emit_v_group, 11)
            put(0, 9, emit_v_group, 12)
            put(0, 10, emit_v_group, 13)
            put(0, 11, emit_qk_group, "q", 1, 0)
            put(0, 12, emit_v_group, 14)
            put(0, 13, emit_qk_group, "k", 1, 0)
            put(0, 14, emit_v_group, 15)
            # block 1: rest of pair-1 k, q chunk for block 2
            put(1, 0, emit_norm, 0)
            put(1, 2, emit_qk_group, "k", 1, 1)
            put(1, 5, emit_qk_group, "k", 1, 2)
            put(1, 9, emit_qk_group, "k", 1, 3)
            put(1, 12, emit_qk_group, "q", 0, 1)
            # blocks 2..7: norms, projections (3+1 split keeps PE slack
            # even), and the remaining q chunks one block ahead of use
            # proj goes at iter >= 5: its explicit dep on the norm multiply
            # (which completes ~3 iterations after the block starts) would
            # otherwise head-of-line-block the in-order PE queue and stall
            # the exp stream at every block boundary.
            put(2, 0, emit_norm, 1)
            put(2, 5, emit_proj, 0)
            put(2, 8, emit_proj, 1)
            put(2, 11, emit_proj, 2)
            put(2, 13, emit_qk_group, "q", 1, 1)
            put(3, 0, emit_norm, 2)
            put(3, 5, emit_proj, 3)
            put(3, 8, emit_qk_group, "q", 0, 2)
            put(4, 0, emit_norm, 3)
            put(4, 5, emit_proj, 4)
            put(4, 8, emit_proj, 5)
            put(4, 11, emit_proj, 6)
            put(4, 13, emit_qk_group, "q", 1, 2)
            put(5, 0, emit_norm, 4)
            put(5, 5, emit_proj, 7)
            put(5, 8, emit_qk_group, "q", 0, 3)
            put(6, 0, emit_norm, 5)
            put(6, 5, emit_proj, 8)
            put(6, 8, emit_proj, 9)
            put(6, 11, emit_proj, 10)
            put(6, 13, emit_qk_group, "q", 1, 3)
            put(7, 0, emit_norm, 6)
            put(7, 5, emit_proj, 11)

            # Minimal prologue: only the chunk-0 q/k of pair 0, then straight
            # into the first score tile.
            emit_qk_group("q", 0, 0)
            emit_qk_group("k", 0, 0)

            # software pipeline: issue ST(si+1) on the PE BEFORE the AV(si)
            # matmuls, and DEFER the j=1 AV matmul by one further iteration.
            # AV(si) j=1 stalls the in-order PE queue on the DVE exp chain
            # (cubic+^32, ~1.4us serial); with ST(si+1) and ST(si+2) both
            # ahead of it in the queue, the array computes the next two score
            # tiles under the exp engines. The next BLOCK's ST(0) is likewise
            # hoisted into the current block's last iteration.
            pend_j1 = [None]

            def emit_av_j1(item):
                av_t, v3s, atts, edep, vcps, start, stop, key = item
                mm_av = nc.tensor.matmul(av_t, v3s, atts, start=start, stop=stop)
                tile.add_dep_helper(mm_av.ins, edep.ins, reason="exp half")
                for cp in vcps:
                    tile.add_dep_helper(mm_av.ins, cp.ins, reason="v tile")
                if stop:
                    av_last[key] = mm_av

            stt = emit_st(blocks[0][1], blocks[0][0], 0)
            for bi, (tq, hp) in enumerate(blocks):
                tsl = slice(tq * 512, (tq + 1) * 512)
                av = [
                    ps_av.tile([2 * HS, 512], F32, tag="av", name=f"av{hp}_{tq}_{j}")
                    for j in range(2)
                ]
                avs[bi] = av
                for si in range(16):
                    st0, st1, par = stt
                    if si in DVE_SI:
                        # split: ScalarE takes head j=0, DVE takes head j=1
                        # (cubic poly of e^t then ^32 by squaring).
                        att_h0 = attp0.tile([P, 512], F16, tag="att0", name="att_h0")
                        att_h1 = attp1.tile([P, 512], F16, tag="att1", name="att_h1")
                        att_h = [att_h0, att_h1]
                        a0 = nc.scalar.activation(att_h0, st0, func=EXP, scale=32.0)
                        scr = dvescr.tile([P, 512], F32, tag="scr")
                        d1 = nc.vector._custom_dve(
                            OP_EXPC,
                            out=scr,
                            in0=st1,
                            s0=EXPC[0],
                            s1=EXPC[1],
                            imm2=EXPC[2],
                        )
                        d2 = nc.vector._custom_dve(OP_SQ5, out=att_h1, in0=scr)
                        exp_dep = (a0, d2)
                        st1_reader[par] = d1
                    else:
                        att = attp.tile([P, 1024], F16, tag="att")
                        att_h = [att[:, 0:512], att[:, 512:1024]]
                        a_j0 = nc.scalar.activation(
                            att[:, 0:512], st0, func=EXP, scale=32.0
                        )
                        a_j1 = nc.scalar.activation(
                            att[:, 512:1024], st1, func=EXP, scale=32.0
                        )
                        exp_dep = (a_j0, a_j1)
                        st1_reader[par] = a_j1
                    if si < 15:
                        stt = emit_st(hp, tq, si + 1)
                    elif bi + 1 < len(blocks):
                        ntq, nhp = blocks[bi + 1]
                        stt = emit_st(nhp, ntq, 0)
                    if pend_j1[0] is not None:
                        emit_av_j1(pend_j1[0])
                        pend_j1[0] = None
                    for fn, args in sched[bi].get(si, ()):
                        fn(*args)
                    v3 = v_sb[si].rearrange("p (h e) -> p h e", h=HPC)
                    mm_av = nc.tensor.matmul(
                        av[0],
                        v3[:, 2 * hp, :],
                        att_h[0],
                        start=(si == 0),
                        stop=(si == 15),
                    )
                    tile.add_dep_helper(mm_av.ins, exp_dep[0].ins, reason="exp half")
                    for cp in v_cp.get(si, ()):
                        tile.add_dep_helper(mm_av.ins, cp.ins, reason="v tile")
                    if si == 15:
                        av_last[(bi, 0)] = mm_av
                    item_j1 = (
                        av[1],
                        v3[:, 2 * hp + 1, :],
                        att_h[1],
                        exp_dep[1],
                        v_cp.get(si, ()),
                        si == 0,
                        si == 15,
                        (bi, 1),
                    )
                    pend_j1[0] = item_j1
            # drain: last block's norm + the final t-chunk's projection.
            # A few dummy matmuls keep the PE out of its low p-state while
            # the cross-engine norm chain runs, so the projection matmuls
            # execute at full clock. The dummies target a ps_st slot (free
            # once the last exp has read it) -- an "av" slot would stall on
            # the final accumulators.
            emit_av_j1(pend_j1[0])
            warm2 = ps_st0.tile([P, 512], F32, tag="st0", name="warm2")
            for _ in range(5):
                nc.tensor.matmul(
                    warm2, warm_in[:, 0:P], warm_in, start=True, stop=True
                )
            emit_norm(7, tail=True)
            for tt in range(12, 16):
                emit_proj(tt)


_NC_CACHE = None


def _combined_act_set_id() -> int:
    """Index (into act_info.json act_func_sets) of a set with exp."""
    try:
        import glob as _glob
        import json as _json
        import neuronxcc

        pat = os.path.join(
            os.path.dirname(neuronxcc.__file__), "pwp", "*", "act_info.json"
        )
        for p in sorted(_glob.glob(pat)):
            sets = _json.load(open(p))["act_func_sets"]
            for i, s in enumerate(sets):
                fns = s.get("act", {})
                if "exp" in fns and "ln" in fns:
                    return i
    except Exception:
        pass
    return 6  # natural_log_exp_and_others in the TRN2 act_info.json


def _dedupe_act_table_loads(nc):
    """Keep one ACT table load; drop the rest.

    Bacc's insert_act_table_loads can thrash (~2.7us per reload). Every
    activation we emit (Exp, Copy) lives in the combined set, so a single
    load up front is sufficient.
    """
    set_id = _combined_act_set_id()
    first = True
    for b in nc.m.functions[0].blocks:
        keep = []
        for inst in b.instructions:
            if isinstance(inst, mybir.InstLoadActFuncSet):
                if first:
                    inst.act_func_set_id = set_id
                    first = False
                    keep.append(inst)
            else:
                keep.append(inst)
        b.instructions[:] = keep


def _build():
    global _NC_CACHE
    if _NC_CACHE is not None:
        return _NC_CACHE
    nc = bacc.Bacc("TRN2", target_bir_lowering=False, debug=False, num_devices=NCORES)
    xT = nc.dram_tensor("xT", [D, T], F16, kind="ExternalInput").ap()
    wq = nc.dram_tensor("wq", [D, HPC * HS], F16, kind="ExternalInput").ap()
    wk = nc.dram_tensor("wk", [D, HPC * HS], F16, kind="ExternalInput").ap()
    wv = nc.dram_tensor("wv", [D, HPC * HS], F16, kind="ExternalInput").ap()
    wo = nc.dram_tensor("wo", [HPC * HS, D], F16, kind="ExternalInput").ap()
    y = nc.dram_tensor("y", [T, D], F16, kind="ExternalOutput").ap()
    with tile.TileContext(nc) as tc:
        _emit(tc, xT, wq, wk, wv, wo, y)
    nc.compile()
    _dedupe_act_table_loads(nc)
    _NC_CACHE = nc
    return nc


def _prep_in_maps(x, Wq, Wk, Wv, Wo, bo):
    x = np.asarray(x, dtype=np.float32)
    Wq = np.asarray(Wq, dtype=np.float32)
    Wk = np.asarray(Wk, dtype=np.float32)
    Wv = np.asarray(Wv, dtype=np.float32)
    Wo = np.asarray(Wo, dtype=np.float32)
    in_maps = []
    for c in range(NCORES):
        b, hh = divmod(c, 2)
        hb = hh * HPC
        in_maps.append(
            {
                "xT": to_fp16(x[b].T),
                # Wq/Wk pre-scaled by 1/16 each: scores come out as
                # score/256, and exp is computed as exp(32*s) on-device.
                "wq": to_fp16(
                    Wq[hb : hb + HPC].transpose(1, 0, 2).reshape(D, HPC * HS) / 16.0
                ),
                "wk": to_fp16(
                    Wk[hb : hb + HPC].transpose(1, 0, 2).reshape(D, HPC * HS) / 16.0
                ),
                "wv": to_fp16(Wv[hb : hb + HPC].transpose(1, 0, 2).reshape(D, HPC * HS)),
                "wo": to_fp16(Wo[hb * HS : (hb + HPC) * HS, :]),
            }
        )
    return in_maps


def _run(in_maps, trace=False):
    nc = _build()
    return run_bass_kernel_spmd(nc, in_maps, list(range(NCORES)), trace=trace)


def _run_prof(in_maps, tmpdir):
    nc = _build()
    return run_bass_kernel_spmd(
        nc, in_maps, list(range(NCORES)), trace=True, tmpdir=tmpdir
    )


def kernel(x, Wq, Wk, Wv, Wo, bo):
    in_maps = _prep_in_maps(x, Wq, Wk, Wv, Wo, bo)
    res = _run(in_maps)
    bo32 = np.asarray(bo, dtype=np.float32)
    y = np.empty((B, T, D), dtype=np.float32)
    for b in range(B):
        y[b] = (
            res.results[2 * b]["y"].astype(np.float32)
            + res.results[2 * b + 1]["y"].astype(np.float32)
            + bo32
        )
    return y


# revision 28
# speedup vs baseline: 1.8407x; 1.8407x over previous
"""Multi-head attention forward on 8 Trainium2 NeuronCores (Bass/Tile).

Problem: B=4, T=2048, D=512, H=8, HS=64, fp32.
  q/k/v = einsum('btd,hde->bhte', x, W{q,k,v})
  att   = softmax(q k^T / sqrt(HS))
  out   = (att v) concat-heads @ Wo + bo

Sharding (8 cores): core c -> batch b=c//2, heads hb=4*(c%2)..hb+4
(data parallel on B x tensor parallel on H). Each core computes its 4 heads'
attention and a partial output projection against its 256 rows of Wo; the
host sums the two partials per batch and adds the bias.

On-device dataflow per core (all matmuls in float16: 1 cycle/row on PE,
~1e-3 rel err; PSUM accumulation is fp32):
  phase 1: qT/kT per head-pair [128=2*HS, T] and v [T, 4*(HS+1)] (ones column
           appended per head for the softmax denominator) from xT [D, T].
           Wq/Wk are pre-scaled by 1/16 each on the host, so raw scores come
           out as score/256 and exp() becomes exp(32*s).
  phase 2: per head-pair, per 512-wide t-chunk, loop s-tiles of 128:
           ST[s,t] matmuls (K=HS=64, two heads row-packed at partitions 0/64),
           then exp over the [128, 1024] PSUM tile SPLIT ACROSS ENGINES:
           ScalarE does head j=0 (and both heads on non-split iterations);
           the DVE does head j=1 via two custom ops (cubic minimax poly of
           e^t on |t|<=0.21, then x^32 by 5 squarings; ~1e-3 max rel err).
           AV matmuls accumulate [65, 512] per head (row 0 = sum of exp).
           Normalize: DVE fast-reciprocal of the sum row, partition-broadcast
           on GpSimd, multiply on DVE -> outT [4*HS, T].
  phase 3: y[t,:] = outT.T @ Wo_rows via accumulating matmuls, ScalarE
           PSUM->SBUF fp16 copy, DMA out. (Bias is added on the host.)

With exp split ~56/72 between DVE and ScalarE, the PE matmul stream
(~330K cycles: ST 131K + AV 131K + qkv 48K + oproj 16K) is the critical
path; the emission schedule keeps it saturated: minimal prologue, all
projection work placed as PE filler with per-tile deadlines, exp engines
balanced so neither ever blocks an AV matmul.
"""
import os
import sys

sys.path.insert(0, "/opt/trn_rl_repo")

import numpy as np
from contextlib import ExitStack

import concourse.bacc as bacc
import concourse.tile as tile
from concourse import mybir
from concourse.bass_utils import run_bass_kernel_spmd
from concourse.dve_spec import Spec, Src0, C0, C1, C2, One, lower
from concourse.dve_uop import DveOpSpec
import concourse.dve_ops as dve_ops
from concourse.dve_ops import DveOp, OPS, _SUB_OPCODE_FOR_NAME, _CUSTOM_DVE_ROW_BASE

B, T, D, H, HS = 4, 2048, 512, 8, 64
NCORES = 8
P = 128
HPC = 4  # heads per core
F32 = mybir.dt.float32
F16 = mybir.dt.float16
EXP = mybir.ActivationFunctionType.Exp

# cubic minimax of e^t on [-0.21, 0.21] (covers |score/8| <= 6.7)
EXPC = (1.00001443, 0.50158342, 0.16650198)
# iterations (within each 16-iteration block) whose j=1 exp half runs on DVE
DVE_SI = frozenset({2, 4, 6, 8, 10, 12, 14})


def _ref_exp_cubic(in0, in1, s0, s1, imm2):
    t = np.asarray(in0, dtype=np.float32)
    return ((np.float32(imm2) * t + np.float32(s1)) * t + np.float32(s0)) * t + np.float32(1)


def _ref_sq5(in0, in1, s0, s1, imm2):
    p = np.asarray(in0, dtype=np.float32)
    for _ in range(5):
        p = p * p
    return p


def _register_exp_ops():
    """Install the two custom DVE ops (idempotent across re-imports)."""
    if hasattr(dve_ops, "EXP_CUBIC_ANT"):
        return dve_ops.EXP_CUBIC_ANT, dve_ops.EXP_SQ5_ANT

    def make(name, body, reference):
        spec = Spec(body=body, reference=reference)
        shas = {}
        for ver in ("v3", "v4"):
            shas[ver] = DveOpSpec(
                name=name, opcode=31, uops=lower(spec, ver=ver), rd1_en=False
            ).sha(ver)
        return DveOp(name, spec, subdim=False, uops_sha=shas)

    t = Src0
    body1 = ((C2 * t + C1) * t + C0) * t + One
    s = Src0
    for _ in range(5):
        s = s * s
    op1 = make("EXP_CUBIC_ANT", body1, _ref_exp_cubic)
    op2 = make("EXP_SQ5_ANT", s, _ref_sq5)
    for op in (op1, op2):
        OPS.append(op)
        _SUB_OPCODE_FOR_NAME[op.name] = _CUSTOM_DVE_ROW_BASE + len(OPS) - 1
    dve_ops.EXP_CUBIC_ANT = op1
    dve_ops.EXP_SQ5_ANT = op2
    return op1, op2


OP_EXPC, OP_SQ5 = _register_exp_ops()


def to_fp16(x: np.ndarray) -> np.ndarray:
    return np.ascontiguousarray(np.asarray(x, dtype=np.float32).astype(np.float16))


def _emit(tc, xT, wq, wk, wv, wo, y):
    nc = tc.nc
    with ExitStack() as ctx:
        persist = ctx.enter_context(tc.tile_pool(name="persist", bufs=1))

        # ---- persistent SBUF tiles ----
        xt_sb = [persist.tile([P, T], F16, tag=f"xt{i}", name=f"xt{i}") for i in range(4)]
        wq_sb = [persist.tile([P, 2 * P], F16, tag=f"wq{i}", name=f"wq{i}") for i in range(4)]
        wk_sb = [persist.tile([P, 2 * P], F16, tag=f"wk{i}", name=f"wk{i}") for i in range(4)]
        wv_sb = [persist.tile([P, 2 * P], F16, tag=f"wv{i}", name=f"wv{i}") for i in range(4)]
        wo_sb = [persist.tile([P, D], F16, tag=f"wo{i}", name=f"wo{i}") for i in range(2)]
        q2 = [persist.tile([P, T], F16, tag=f"q2{i}", name=f"q2_{i}") for i in range(2)]
        k2 = [persist.tile([P, T], F16, tag=f"k2{i}", name=f"k2_{i}") for i in range(2)]
        v_sb = [persist.tile([P, HPC * 2 * HS], F16, tag=f"v{i}", name=f"v{i}") for i in range(16)]
        out2 = [persist.tile([P, T], F16, tag=f"o2{i}", name=f"o2_{i}") for i in range(2)]
        ones_v16 = persist.tile([P, HPC], F16, tag="ones_v16")
        warm_in = persist.tile([P, 512], F16, tag="warm_in")

        # PE warm-up: ~3.5us of dummy matmuls during the DMA lead-in flips
        # the HAM clock gate to 2.4 GHz before the real matmuls start.
        with tc.tile_pool(name="ps_warm", bufs=1, space="PSUM") as ps_warm:
            nc.vector.memset(warm_in, 0.5)
            wp = ps_warm.tile([P, 512], F32, tag="warm")
            for _ in range(8):
                nc.tensor.matmul(wp, warm_in[:, 0:P], warm_in, start=True, stop=True)

        # Input DMAs on the three DMA-capable queues (sync/scalar/gpsimd).
        # The four chunk-0 xT tiles lead each queue so the first projection
        # group can start ~2.5us after issue; everything else is ordered by
        # its first consumer's deadline. The scalar queue is kept short so
        # the ACT sequencer is free before the first exp.
        c0 = slice(0, 512)

        def dsl(i):
            return slice(i * P, (i + 1) * P)

        nc.sync.dma_start(out=xt_sb[0][:, c0], in_=xT[dsl(0), c0])
        nc.scalar.dma_start(out=xt_sb[2][:, c0], in_=xT[dsl(2), c0])
        nc.gpsimd.dma_start(out=xt_sb[3][:, c0], in_=xT[dsl(3), c0])
        nc.sync.dma_start(out=xt_sb[1][:, c0], in_=xT[dsl(1), c0])
        nc.scalar.dma_start(out=wq_sb[0], in_=wq[dsl(0), :])
        nc.scalar.dma_start(out=wq_sb[1], in_=wq[dsl(1), :])
        nc.sync.dma_start(out=wq_sb[2], in_=wq[dsl(2), :])
        nc.sync.dma_start(out=wq_sb[3], in_=wq[dsl(3), :])
        for i in range(4):
            nc.gpsimd.dma_start(out=wk_sb[i], in_=wk[dsl(i), :])
        nc.sync.dma_start(out=wv_sb[0], in_=wv[dsl(0), :])
        nc.sync.dma_start(out=wv_sb[1], in_=wv[dsl(1), :])
        nc.gpsimd.dma_start(out=wv_sb[2], in_=wv[dsl(2), :])
        nc.gpsimd.dma_start(out=wv_sb[3], in_=wv[dsl(3), :])
        for tch in range(1, 4):
            csl = slice(tch * 512, (tch + 1) * 512)
            for i in range(4):
                nc.sync.dma_start(out=xt_sb[i][:, csl], in_=xT[dsl(i), csl])
        for i in range(2):
            nc.gpsimd.dma_start(out=wo_sb[i], in_=wo[i * P : (i + 1) * P, :])
        nc.vector.memset(ones_v16, 1.0)

        # One shared PSUM layout: st 2x[128,1024] (4 banks) + 4 general
        # [128,512] slots (tag "av": projection groups AND the AV
        # accumulators) = 8 banks.
        with (
            tc.tile_pool(name="ps_st0", bufs=2, space="PSUM") as ps_st0,
            tc.tile_pool(name="ps_st1", bufs=2, space="PSUM") as ps_st1,
            tc.tile_pool(name="ps_av", bufs=4, space="PSUM") as ps_av,
            tc.tile_pool(name="attp", bufs=6) as attp,
            tc.tile_pool(name="attp0", bufs=4) as attp0,
            tc.tile_pool(name="attp1", bufs=4) as attp1,
            tc.tile_pool(name="dvescr", bufs=3) as dvescr,
            tc.tile_pool(name="nrm", bufs=6) as nrm,
            tc.tile_pool(name="yout", bufs=3) as yout,
        ):
            blocks = [(tq, hp) for tq in range(4) for hp in range(2)]
            avs = [None] * len(blocks)
            # Explicit cross-engine deps where Tile's subtile tracking is
            # unreliable (partition-subrange reads of DVE-written tiles):
            # q/k chunk copies -> ST matmuls, norm multiplies -> projection.
            qk_cp = {}
            nrm_mul = {}
            av_last = {}
            v_cp = {}

            def emit_qk_group(kind, pr, tch):
                # one [128,512] chunk of the q or k projection (4 accumulating
                # matmuls over D, then a cast-copy to fp16 SBUF)
                w_sb, dst = (wq_sb, q2) if kind == "q" else (wk_sb, k2)
                psl = slice(pr * P, (pr + 1) * P)
                tsl = slice(tch * 512, (tch + 1) * 512)
                pt = ps_av.tile(
                    [P, 512], F32, tag="av", name=f"qk{kind}_{pr}_{tch}"
                )
                for di in range(4):
                    nc.tensor.matmul(
                        pt,
                        w_sb[di][:, psl],
                        xt_sb[di][:, tsl],
                        start=(di == 0),
                        stop=(di == 3),
                    )
                qk_cp[(kind, pr, tch)] = nc.vector.tensor_copy(dst[pr][:, tsl], pt)

            def emit_v_group(tt):
                ttsl = slice(tt * P, (tt + 1) * P)
                pv = ps_av.tile([P, 2 * P], F32, tag="av", name=f"pv{tt}")
                for di in range(4):
                    nc.tensor.matmul(
                        pv,
                        xt_sb[di][:, ttsl],
                        wv_sb[di],
                        start=(di == 0),
                        stop=(di == 3),
                    )
                # Per head, the 128 lhsT columns are [ones, 63 junk, v x 64]:
                # the denominator accumulates on partition 0 of the AV tile
                # (read directly by the DVE reciprocal -- the custom op
                # mishandles nonzero partition offsets) and the numerators
                # land at partition 64 (quadrant-aligned for the multiply).
                # The junk columns cost nothing: matmul time depends only on
                # the moving free size, and their accumulator rows are never
                # read.
                v3 = v_sb[tt].rearrange("p (h e) -> p h e", h=HPC)
                c1 = nc.vector.tensor_copy(
                    v3[:, :, HS : 2 * HS], pv.rearrange("p (h e) -> p h e", h=HPC)
                )
                c2 = nc.vector.tensor_copy(v3[:, :, 0], ones_v16)
                v_cp[tt] = (c1, c2)

            def emit_proj(tt):
                # output projection t-tile; PSUM->SBUF fp16 copy on ScalarE
                # (bias is added by the host after the partial-sum gather)
                ttsl = slice(tt * P, (tt + 1) * P)
                tq = tt // 4
                yp = ps_av.tile([P, D], F32, tag="av", name=f"yp{tt}")
                for hp in range(2):
                    mm = nc.tensor.matmul(
                        yp, out2[hp][:, ttsl], wo_sb[hp], start=(hp == 0), stop=(hp == 1)
                    )
                    for j in range(2):
                        dep = nrm_mul.get((hp, tq, j))
                        if dep is not None:
                            tile.add_dep_helper(mm.ins, dep.ins, reason="norm ready")
                ys = yout.tile([P, D], F16, tag="y")
                nc.scalar.copy(ys, yp)
                nc.sync.dma_start(out=y[ttsl, :], in_=ys)

            def emit_norm(b, tail=False):
                # divide the pair's unnormalized outputs by their sums of
                # exp. First drain each accumulator PSUM->SBUF with one DVE
                # copy -- this frees the PSUM slot ~0.8us after the last AV
                # matmul instead of after the whole norm chain, so the next
                # block's accumulators aren't WAR-gated on it. Then: DVE
                # fast reciprocal of the sum row, partition-broadcast on
                # GpSimd, multiply on DVE, all reading the SBUF copy.
                tq, hp = blocks[b]
                av = avs[b]
                tsl = slice(tq * 512, (tq + 1) * 512)
                rcs = []
                for j in range(2):
                    fin = av_last[(b, j)]
                    rc = nrm.tile([1, 512], F32, tag="rc", name=f"rc{b}_{j}")
                    ri = nc.vector.reciprocal_approx_fast(rc, av[j][0:1, :])
                    tile.add_dep_helper(ri.ins, fin.ins, reason="av accum done")
                    rcs.append(rc)
                for j in range(2):
                    fin = av_last[(b, j)]
                    bco = nrm.tile([HS, 512], F32, tag="bco", name=f"bco{b}_{j}")
                    nc.gpsimd.partition_broadcast(bco, rcs[j])
                    mi = nc.vector.tensor_mul(
                        out2[hp][j * HS : (j + 1) * HS, tsl],
                        av[j][HS : 2 * HS, :],
                        bco,
                    )
                    tile.add_dep_helper(mi.ins, fin.ins, reason="av accum done")
                    nrm_mul[(hp, tq, j)] = mi

            # last exp instruction that read each ps_st1 buffer, by parity:
            # its completion gates the buffer's reuse two iterations later.
            st1_reader = {}
            st_parity = [0]

            def emit_st(hp, tq, si):
                # scores for both heads of the pair, row-packed at
                # partitions 0 / 64 (K=64 each). One PSUM tile PER HEAD so
                # each has exactly ONE exp reader (two readers of one PSUM
                # tile fully serialize in Tile's tracking). The st1 buffer's
                # WAR dep is added explicitly to the FIRST matmul of the
                # pair: Tile then elides the auto-WAR wait on the second,
                # which lets it overlap the first in the other array
                # row-group (~free instead of ~320ns).
                tsl = slice(tq * 512, (tq + 1) * 512)
                ssl = slice(si * P, (si + 1) * P)
                st0 = ps_st0.tile([P, 512], F32, tag="st0", name=f"st0_{hp}_{tq}_{si}")
                st1 = ps_st1.tile([P, 512], F32, tag="st1", name=f"st1_{hp}_{tq}_{si}")
                par = st_parity[0]
                st_parity[0] ^= 1
                prev_rd = st1_reader.get(par)
                for j, stj in enumerate((st0, st1)):
                    hsl = slice(j * HS, (j + 1) * HS)
                    mm = nc.tensor.matmul(
                        stj,
                        k2[hp][hsl, ssl],
                        q2[hp][hsl, tsl],
                        start=True,
                        stop=True,
                    )
                    if j == 0 and prev_rd is not None:
                        tile.add_dep_helper(mm.ins, prev_rd.ins, reason="st1 war")
                    for key in (("q", hp, tq), ("k", hp, si // 4)):
                        cp = qk_cp.get(key)
                        if cp is not None:
                            tile.add_dep_helper(mm.ins, cp.ins, reason="qk chunk")
                return (st0, st1, par)

            # Filler schedule: sched[b][si] = list of thunks issued on the PE
            # (or DVE/GpSimd for norms) inside iteration si of block b,
            # between the hoisted ST(si+1) and the AV(si) matmuls. Every item
            # is placed ahead of its consumer's deadline:
            #   k chunk c of the running pair  -> before ST(4c) issues (iter 4c-1)
            #   v tile si                      -> before AV(si) (iter si)
            #   q/k chunk 0 of the next pair   -> before the next block's ST(0)
            #   q chunk tq                     -> before block (tq,hp) starts
            #   norm of block b-1              -> iter 0 (frees its PSUM pair)
            #   proj tiles of t-chunk tq       -> after both norms of tq
            sched = [dict() for _ in range(len(blocks))]

            def put(b, it, fn, *args):
                sched[b].setdefault(it, []).append((fn, args))

            # block 0: v tiles + remaining k chunks of pair 0, then pair 1
            put(0, 0, emit_v_group, 0)
            put(0, 0, emit_v_group, 1)
            put(0, 0, emit_qk_group, "k", 0, 1)
            put(0, 1, emit_v_group, 2)
            put(0, 1, emit_v_group, 3)
            put(0, 2, emit_v_group, 4)
            put(0, 3, emit_qk_group, "k", 0, 2)
            put(0, 3, emit_v_group, 5)
            put(0, 4, emit_v_group, 6)
            put(0, 4, emit_v_group, 7)
            put(0, 5, emit_v_group, 8)
            put(0, 6, emit_v_group, 9)
            put(0, 7, emit_qk_group, "k", 0, 3)
            put(0, 7, emit_v_group, 10)
            put(0, 8, emit_v_group, 11)
            put(0, 9, emit_v_group, 12)
            put(0, 10, emit_v_group, 13)
            put(0, 11, emit_qk_group, "q", 1, 0)
            put(0, 12, emit_v_group, 14)
            put(0, 13, emit_qk_group, "k", 1, 0)
            put(0, 14, emit_v_group, 15)
            # block 1: rest of pair-1 k, q chunk for block 2
            put(1, 0, emit_norm, 0)
            put(1, 2, emit_qk_group, "k", 1, 1)
            put(1, 5, emit_qk_group, "k", 1, 2)
            put(1, 9, emit_qk_group, "k", 1, 3)
            put(1, 12, emit_qk_group, "q", 0, 1)
            # blocks 2..7: norms, projections (3+1 split keeps PE slack
            # even), and the remaining q chunks one block ahead of use
            # proj goes at iter >= 5: its explicit dep on the norm multiply
            # (which completes ~3 iterations after the block starts) would
            # otherwise head-of-line-block the in-order PE queue and stall
            # the exp stream at every block boundary.
            put(2, 0, emit_norm, 1)
            put(2, 5, emit_proj, 0)
            put(2, 8, emit_proj, 1)
            put(2, 11, emit_proj, 2)
            put(2, 13, emit_qk_group, "q", 1, 1)
            put(3, 0, emit_norm, 2)
            put(3, 5, emit_proj, 3)
            put(3, 8, emit_qk_group, "q", 0, 2)
            put(4, 0, emit_norm, 3)
            put(4, 5, emit_proj, 4)
            put(4, 8, emit_proj, 5)
            put(4, 11, emit_proj, 6)
            put(4, 13, emit_qk_group, "q", 1, 2)
            put(5, 0, emit_norm, 4)
            put(5, 5, emit_proj, 7)
            put(5, 8, emit_qk_group, "q", 0, 3)
            put(6, 0, emit_norm, 5)
            put(6, 5, emit_proj, 8)
            put(6, 8, emit_proj, 9)
            put(6, 11, emit_proj, 10)
            put(6, 13, emit_qk_group, "q", 1, 3)
            put(7, 0, emit_norm, 6)
            put(7, 5, emit_proj, 11)

            # Minimal prologue: only the chunk-0 q/k of pair 0, then straight
            # into the first score tile.
            emit_qk_group("q", 0, 0)
            emit_qk_group("k", 0, 0)

            # software pipeline: issue ST(si+1) on the PE BEFORE the AV(si)
            # matmuls, and DEFER the j=1 AV matmul by one further iteration.
            # AV(si) j=1 stalls the in-order PE queue on the DVE exp chain
            # (cubic+^32, ~1.4us serial); with ST(si+1) and ST(si+2) both
            # ahead of it in the queue, the array computes the next two score
            # tiles under the exp engines. The next BLOCK's ST(0) is likewise
            # hoisted into the current block's last iteration.
            pend_j1 = [None]

            def emit_av_j1(item):
                av_t, v3s, atts, edep, vcps, start, stop, key = item
                mm_av = nc.tensor.matmul(av_t, v3s, atts, start=start, stop=stop)
                tile.add_dep_helper(mm_av.ins, edep.ins, reason="exp half")
                for cp in vcps:
                    tile.add_dep_helper(mm_av.ins, cp.ins, reason="v tile")
                if stop:
                    av_last[key] = mm_av

            stt = emit_st(blocks[0][1], blocks[0][0], 0)
            for bi, (tq, hp) in enumerate(blocks):
                tsl = slice(tq * 512, (tq + 1) * 512)
                av = [
                    ps_av.tile([2 * HS, 512], F32, tag="av", name=f"av{hp}_{tq}_{j}")
                    for j in range(2)
                ]
                avs[bi] = av
                for si in range(16):
                    st0, st1, par = stt
                    if si in DVE_SI:
                        # split: ScalarE takes head j=0, DVE takes head j=1
                        # (cubic poly of e^t then ^32 by squaring).
                        att_h0 = attp0.tile([P, 512], F16, tag="att0", name="att_h0")
                        att_h1 = attp1.tile([P, 512], F16, tag="att1", name="att_h1")
                        att_h = [att_h0, att_h1]
                        a0 = nc.scalar.activation(att_h0, st0, func=EXP, scale=32.0)
                        scr = dvescr.tile([P, 512], F32, tag="scr")
                        d1 = nc.vector._custom_dve(
                            OP_EXPC,
                            out=scr,
                            in0=st1,
                            s0=EXPC[0],
                            s1=EXPC[1],
                            imm2=EXPC[2],
                        )
                        d2 = nc.vector._custom_dve(OP_SQ5, out=att_h1, in0=scr)
                        exp_dep = (a0, d2)
                        st1_reader[par] = d1
                    else:
                        att = attp.tile([P, 1024], F16, tag="att")
                        att_h = [att[:, 0:512], att[:, 512:1024]]
                        a_j0 = nc.scalar.activation(
                            att[:, 0:512], st0, func=EXP, scale=32.0
                        )
                        a_j1 = nc.scalar.activation(
                            att[:, 512:1024], st1, func=EXP, scale=32.0
                        )
                        exp_dep = (a_j0, a_j1)
                        st1_reader[par] = a_j1
                    if si < 15:
                        stt = emit_st(hp, tq, si + 1)
                    elif bi + 1 < len(blocks):
                        ntq, nhp = blocks[bi + 1]
                        stt = emit_st(nhp, ntq, 0)
                    if pend_j1[0] is not None:
                        emit_av_j1(pend_j1[0])
                        pend_j1[0] = None
                    for fn, args in sched[bi].get(si, ()):
                        fn(*args)
                    v3 = v_sb[si].rearrange("p (h e) -> p h e", h=HPC)
                    mm_av = nc.tensor.matmul(
                        av[0],
                        v3[:, 2 * hp, :],
                        att_h[0],
                        start=(si == 0),
                        stop=(si == 15),
                    )
                    tile.add_dep_helper(mm_av.ins, exp_dep[0].ins, reason="exp half")
                    for cp in v_cp.get(si, ()):
                        tile.add_dep_helper(mm_av.ins, cp.ins, reason="v tile")
                    if si == 15:
                        av_last[(bi, 0)] = mm_av
                    item_j1 = (
                        av[1],
                        v3[:, 2 * hp + 1, :],
                        att_h[1],
                        exp_dep[1],
                        v_cp.get(si, ()),
                        si == 0,
                        si == 15,
                        (bi, 1),
                    )
                    pend_j1[0] = item_j1
            # drain: last block's norm + the final t-chunk's projection.
            # A few dummy matmuls keep the PE out of its low p-state while
            # the cross-engine norm chain runs, so the projection matmuls
            # execute at full clock. The dummies target a ps_st slot (free
            # once the last exp has read it) -- an "av" slot would stall on
            # the final accumulators.
            emit_av_j1(pend_j1[0])
            warm2 = ps_st0.tile([P, 512], F32, tag="st0", name="warm2")
            for _ in range(5):
                nc.tensor.matmul(
                    warm2, warm_in[:, 0:P], warm_in, start=True, stop=True
                )
            emit_norm(7, tail=True)
            for tt in range(12, 16):
                emit_proj(tt)


_NC_CACHE = None


def _combined_act_set_id() -> int:
    """Index (into act_info.json act_func_sets) of a set with exp."""
    try:
        import glob as _glob
        import json as _json
        import neuronxcc

        pat = os.path.join(
            os.path.dirname(neuronxcc.__file__), "pwp", "*", "act_info.json"
        )
        for p in sorted(_glob.glob(pat)):
            sets = _json.load(open(p))["act_func_sets"]
            for i, s in enumerate(sets):
                fns = s.get("act", {})
                if "exp" in fns and "ln" in fns:
                    return i
    except Exception:
        pass
    return 6  # natural_log_exp_and_others in the TRN2 act_info.json


def _dedupe_act_table_loads(nc):
    """Keep one ACT table load; drop the rest.

    Bacc's insert_act_table_loads can thrash (~2.7us per reload). Every
    activation we emit (Exp, Copy) lives in the combined set, so a single
    load up front is sufficient.
    """
    set_id = _combined_act_set_id()
    first = True
    for b in nc.m.functions[0].blocks:
        keep = []
        for inst in b.instructions:
            if isinstance(inst, mybir.InstLoadActFuncSet):
                if first:
                    inst.act_func_set_id = set_id
                    first = False
                    keep.append(inst)
            else:
                keep.append(inst)
        b.instructions[:] = keep


def _build():
    global _NC_CACHE
    if _NC_CACHE is not None:
        return _NC_CACHE
    nc = bacc.Bacc("TRN2", target_bir_lowering=False, debug=False, num_devices=NCORES)
    xT = nc.dram_tensor("xT", [D, T], F16, kind="ExternalInput").ap()
    wq = nc.dram_tensor("wq", [D, HPC * HS], F16, kind="ExternalInput").ap()
    wk = nc.dram_tensor("wk", [D, HPC * HS], F16, kind="ExternalInput").ap()
    wv = nc.dram_tensor("wv", [D, HPC * HS], F16, kind="ExternalInput").ap()
    wo = nc.dram_tensor("wo", [HPC * HS, D], F16, kind="ExternalInput").ap()
    y = nc.dram_tensor("y", [T, D], F16, kind="ExternalOutput").ap()
    with tile.TileContext(nc) as tc:
        _emit(tc, xT, wq, wk, wv, wo, y)
    nc.compile()
    _dedupe_act_table_loads(nc)
    _NC_CACHE = nc
    return nc


def _prep_in_maps(x, Wq, Wk, Wv, Wo, bo):
    x = np.asarray(x, dtype=np.float32)
    Wq = np.asarray(Wq, dtype=np.float32)
    Wk = np.asarray(Wk, dtype=np.float32)
    Wv = np.asarray(Wv, dtype=np.float32)
    Wo = np.asarray(Wo, dtype=np.float32)
    in_maps = []
    for c in range(NCORES):
        b, hh = divmod(c, 2)
        hb = hh * HPC
        in_maps.append(
            {
                "xT": to_fp16(x[b].T),
                # Wq/Wk pre-scaled by 1/16 each: scores come out as
                # score/256, and exp is computed as exp(32*s) on-device.
                "wq": to_fp16(
                    Wq[hb : hb + HPC].transpose(1, 0, 2).reshape(D, HPC * HS) / 16.0
                ),
                "wk": to_fp16(
                    Wk[hb : hb + HPC].transpose(1, 0, 2).reshape(D, HPC * HS) / 16.0
                ),
                "wv": to_fp16(Wv[hb : hb + HPC].transpose(1, 0, 2).reshape(D, HPC * HS)),
                "wo": to_fp16(Wo[hb * HS : (hb + HPC) * HS, :]),
            }
        )
    return in_maps


def _run(in_maps, trace=False):
    nc = _build()
    return run_bass_kernel_spmd(nc, in_maps, list(range(NCORES)), trace=trace)


def _run_prof(in_maps, tmpdir):
    nc = _build()
    return run_bass_kernel_spmd(
        nc, in_maps, list(range(NCORES)), trace=True, tmpdir=tmpdir
    )


def kernel(x, Wq, Wk, Wv, Wo, bo):
    in_maps = _prep_in_maps(x, Wq, Wk, Wv, Wo, bo)
    res = _run(in_maps)
    bo32 = np.asarray(bo, dtype=np.float32)
    y = np.empty((B, T, D), dtype=np.float32)
    for b in range(B):
        y[b] = (
            res.results[2 * b]["y"].astype(np.float32)
            + res.results[2 * b + 1]["y"].astype(np.float32)
            + bo32
        )
    return y
